# revision 21
# baseline (speedup 1.0000x reference)
"""Trainium2 Bass kernel for nn_Decoder (ragged expand + 4-layer decoder).

Sharding: 8 cores = 4 pairs. Pair p handles batch b=p (data parallel over B).
Within a pair, Megatron TP-2: wq/wk/wv/w1/w3 column-sharded, wo/w2 row-sharded,
pair AllGather after attention, AllReduce after FFN.

Schedule is chunk-pipelined so collectives overlap with dense matmul work:
  attn(ch0) -> AG0 kick -> attn(ch1) -> AG1 kick -> wo(ch0) -> ffn(ch0)
  -> AR0 kick -> wo(ch1) -> ffn(ch1) -> AR1 kick -> [next layer] AR0
  retrieve -> rms/qkv/attn(ch0) -> AR1 retrieve -> ...
This keeps the PE busy across every collective (and keeps the HAM clock
gate at 8/8).

Device layout: activations live transposed in SBUF as xT[d, l] (partition=d),
tiled [128, n_dtiles, L]. All matmuls bf16 inputs, fp32 PSUM accumulate;
residual stream fp32.
"""

import numpy as np
import ml_dtypes

B, L, D, KB = 4, 1024, 1024, 128
H, HD, NL, FF = 16, 64, 4, 4096
EPS = 1e-5
NCORES = 8
TP = 2
DQ = D // TP      # 512 local q/k/v dims (8 heads)
DF = FF // TP     # 2048 local ffn dims
NH = H // TP      # 8 local heads
CH = 512          # sequence chunk for moving dim
NCH = L // CH     # 2
NDT = D // 128    # 8 d-tiles
NQT = DQ // 128   # 4 local qkv tiles
NFT = DF // 128   # 16 local ffn tiles
NLT = L // 128    # 8 sequence tiles

_CACHE = {}
# debug/bisect toggles (consulted at build time)
OPTS = {
    "recip_approx": True,   # reciprocal_approx_fast vs vector.reciprocal
    "cc_shared": False,
    "ffn_cc_bf16": True,
    "dve_copies": True,     # DVE tensor_copy vs scalar activation Copy
    "rope_psum": True,      # rope muls read PSUM directly vs qsb staging
    "dmask_mult": True,     # DVE causal-mask multiply vs gpsimd affine_select
    "pack_scores": True,    # head-pair score matmuls on 64-row PE tiles
    "lnexp_recip": True,    # softmax 1/denom via scalar Exp(-Ln(x))
    "two_psum": False,      # rejected: DVE reads at most one PSUM operand
    "sync_fetch": True,     # collective-result DMAs on sync queue (gpsimd
                            # queue head-of-line-blocks behind collectives)
}


def _build(pairs=4):
    import concourse.mybir as mybir
    import concourse.tile as tile
    from concourse import bacc

    f32 = mybir.dt.float32
    bf = mybir.dt.bfloat16
    Alu = mybir.AluOpType
    Act = mybir.ActivationFunctionType

    nc = bacc.Bacc(
        "TRN2", target_bir_lowering=False, debug=False, num_devices=2 * pairs
    )
    sim_mode = pairs == 1  # CoreSim lacks Silu; decompose as x*sigmoid(x)

    # ---- DRAM I/O (per core) ----
    xp_d = nc.dram_tensor("xp", [KB, D], f32, kind="ExternalInput")
    bnd_d = nc.dram_tensor("bnd", [KB, 1], f32, kind="ExternalInput")
    bnds_d = nc.dram_tensor("bnds", [KB, 1], f32, kind="ExternalInput")
    xres_d = nc.dram_tensor("xres", [128, NDT, L], f32, kind="ExternalInput")
    cos_d = nc.dram_tensor("cosr", [128, L], bf, kind="ExternalInput")
    sin_d = nc.dram_tensor("sinr", [128, L], bf, kind="ExternalInput")
    wq_d = nc.dram_tensor("wq", [NL, 128, NDT, DQ], bf, kind="ExternalInput")
    wk_d = nc.dram_tensor("wk", [NL, 128, NDT, DQ], bf, kind="ExternalInput")
    wv_d = nc.dram_tensor("wv", [NL, 128, NDT, DQ], bf, kind="ExternalInput")
    wo_d = nc.dram_tensor("wo", [NL, 128, NDT, D], bf, kind="ExternalInput")
    w1_d = nc.dram_tensor("w1", [NL, 128, NDT, DF], bf, kind="ExternalInput")
    w3_d = nc.dram_tensor("w3", [NL, 128, NDT, DF], bf, kind="ExternalInput")
    w2_d = nc.dram_tensor("w2", [NL, 128, NFT, D], bf, kind="ExternalInput")
    onec_d = nc.dram_tensor("onec", [128, 1], bf, kind="ExternalInput")
    eps_d = nc.dram_tensor("epsc", [1, 1], f32, kind="ExternalInput")
    oner_d = nc.dram_tensor("oner", [1, 128], bf, kind="ExternalInput")
    nwr_d = nc.dram_tensor("nwr", [1, D], bf, kind="ExternalInput")
    rsg_d = nc.dram_tensor("rsg", [128, 128], bf, kind="ExternalInput")
    dmask_d = nc.dram_tensor("dmask", [128, 128], bf, kind="ExternalInput")
    out_d = nc.dram_tensor("out", [D, L], f32, kind="ExternalOutput")

    groups = [[2 * p, 2 * p + 1] for p in range(pairs)]
    adsp = "Shared" if OPTS["cc_shared"] else "Local"
    ccdt = bf if OPTS["ffn_cc_bf16"] else f32

    with tile.TileContext(nc) as tc:
        with (
            tc.tile_pool(name="persist", bufs=1) as pp,
            tc.tile_pool(name="gbig", bufs=1) as gp,
            tc.tile_pool(name="wres", bufs=1) as wp,
            tc.tile_pool(name="wstream", bufs=3) as ws,
            tc.tile_pool(name="small", bufs=3) as sp,
            tc.tile_pool(name="rows", bufs=2) as rp,
            tc.tile_pool(name="stage", bufs=2) as st,
            tc.tile_pool(name="psA", bufs=2, space="PSUM") as psA,
            tc.tile_pool(name="psB", bufs=2, space="PSUM") as psB,
            tc.tile_pool(name="psC", bufs=2, space="PSUM") as psC,
            tc.tile_pool(name="psD", bufs=1, space="PSUM") as psD,
            tc.tile_pool(name="psE", bufs=1, space="PSUM") as psE,
            tc.tile_pool(name="dram", bufs=1, space="DRAM") as dp,
        ):
            # ---- persistent tiles ----
            xT = pp.tile([128, NDT, L], f32)
            hh = pp.tile([128, NDT, L], bf)
            qT = pp.tile([128, NQT, L], bf)
            kT = pp.tile([128, NQT, L], bf)
            vS = pp.tile([128, NLT, NH * 65], bf)
            oT = pp.tile([128, NQT, L], bf)
            cosr = pp.tile([128, L], bf)
            sinr = pp.tile([128, L], bf)
            onec = pp.tile([128, 1], bf)
            epsc = pp.tile([1, 1], f32)
            oner = pp.tile([1, 128], bf)
            nwr = pp.tile([1, D], bf)
            rsg = pp.tile([128, 128], bf)
            dmask = pp.tile([128, 128], bf)
            bnd = pp.tile([KB, 1], f32)
            bnds = pp.tile([KB, 1], f32)
            # big scratch region (tag-shared): g for FFN, out staging at end
            g = gp.tile([128, NFT, L], bf, tag="big")

            nc.sync.dma_start(cosr[:], cos_d[:])
            nc.sync.dma_start(sinr[:], sin_d[:])
            nc.sync.dma_start(onec[:], onec_d[:])
            nc.sync.dma_start(epsc[:], eps_d[:])
            nc.sync.dma_start(oner[:], oner_d[:])
            nc.sync.dma_start(nwr[:], nwr_d[:])
            nc.sync.dma_start(rsg[:], rsg_d[:])
            nc.sync.dma_start(dmask[:], dmask_d[:])
            nc.sync.dma_start(bnd[:], bnd_d[:])
            nc.sync.dma_start(bnds[:], bnds_d[:])
            nc.sync.dma_start(xT[:], xres_d[:])

            vS_r = vS.rearrange("p t (h e) -> p t h e", e=65)
            nc.gpsimd.memset(vS_r[:, :, :, 64:65], 1.0)

            # ---- ragged expand: xT += one_hot_gather(x_processed) ----
            # setup scratch shares the FFN g region (disjoint lifetimes)
            setup = gp.tile([128, 4, L], f32, tag="big")
            iota, cmp0, gt, xp = (setup[:, i, :] for i in range(4))
            nc.gpsimd.iota(
                iota, pattern=[[1, L]], base=0, channel_multiplier=0,
                allow_small_or_imprecise_dtypes=True,
            )
            nc.vector.tensor_scalar(cmp0, iota, bnd[:], None, Alu.is_ge)
            # gt = (l >= b_j) - (l >= b_{j+1}), computed via two tensor_scalar
            nc.vector.tensor_scalar(gt, iota, bnds[:], None, Alu.is_ge)
            nc.vector.tensor_sub(gt, cmp0, gt)
            nc.sync.dma_start(xp, xp_d[:])
            for dt in range(NDT):
                for ch in range(NCH):
                    cs = slice(ch * CH, (ch + 1) * CH)
                    pool, tg = (psA, "A") if ch else (psC, "C")
                    ps = pool.tile([128, CH], f32, tag=tg, name="pse")
                    nc.tensor.matmul(
                        ps[:], xp[:, dt * 128:(dt + 1) * 128], gt[:, cs],
                        start=True, stop=True,
                    )
                    nc.vector.tensor_add(xT[:, dt, cs], xT[:, dt, cs], ps[:])

            # ---- helpers ----
            def rmsnorm_ch(dest, ch, with_nw=False):
                """dest(ch) = xT * rsqrt(mean(xT^2)+eps) [* norm_w].

                Squares on DVE and rsqrt as exp(-0.5*ln(x)) keep the scalar
                engine pinned to the Exp/Ln activation-table set — Square/
                Sqrt here would thrash table reloads against the softmax
                Exps running concurrently on the scalar queue."""
                cs = slice(ch * CH, (ch + 1) * CH)
                ssp = psD.tile([1, CH], f32, tag="D")
                for dt in range(NDT):
                    sq = sp.tile([128, CH], bf, tag="sc3")
                    nc.vector.tensor_mul(sq[:], xT[:, dt, cs], xT[:, dt, cs])
                    nc.tensor.matmul(
                        ssp[:], onec[:], sq[:],
                        start=(dt == 0), stop=(dt == NDT - 1),
                    )
                srow = rp.tile([1, CH], f32, tag="srow")
                nc.scalar.activation(
                    srow[:], ssp[:], Act.Ln, bias=epsc[:], scale=1.0 / D
                )
                rrow = rp.tile([1, CH], bf, tag="rbf")
                nc.scalar.activation(rrow[:], srow[:], Act.Exp, scale=-0.5)
                if not with_nw:
                    bps = psE.tile([128, CH], f32, tag="E")
                    nc.tensor.matmul(
                        bps[:], oner[:], rrow[:], start=True, stop=True
                    )
                    for dt in range(NDT):
                        nc.vector.tensor_mul(
                            dest[:, dt, cs], xT[:, dt, cs], bps[:]
                        )
                else:
                    for dt in range(NDT):
                        bps = psE.tile([128, CH], f32, tag="E")
                        nc.tensor.matmul(
                            bps[:], nwr[:, dt * 128:(dt + 1) * 128],
                            rrow[:], start=True, stop=True,
                        )
                        nc.vector.tensor_mul(
                            dest[:, dt, cs], xT[:, dt, cs], bps[:]
                        )

            pools8 = (psA, psA, psB, psB, psC, psC, psD, psE)
            tags8 = ("A", "A", "B", "B", "C", "C", "D", "E")

            fetch_eng = nc.sync if OPTS["sync_fetch"] else nc.gpsimd

            def retrieve_ar(outb, ch):
                cs = slice(ch * CH, (ch + 1) * CH)
                for ot in range(NDT):
                    ret = st.tile([128, CH], ccdt, tag="ret")
                    fetch_eng.dma_start(ret[:], outb[:, ot, :])
                    nc.vector.tensor_add(
                        xT[:, ot, cs], xT[:, ot, cs], ret[:]
                    )

            pending_ar = [None, None]

            # ==================== layers ====================
            for ly in range(NL):
                wq = wp.tile([128, NDT, DQ], bf, tag="wq")
                wk = wp.tile([128, NDT, DQ], bf, tag="wk")
                wv = wp.tile([128, NDT, DQ], bf, tag="wv")
                wo = wp.tile([128, NDT, D], bf, tag="wo")
                nc.sync.dma_start(wq[:], wq_d[ly])
                nc.sync.dma_start(wk[:], wk_d[ly])
                nc.sync.dma_start(wv[:], wv_d[ly])
                nc.sync.dma_start(wo[:], wo_d[ly])

                ag_bufs = []
                for ch in range(NCH):
                    cs = slice(ch * CH, (ch + 1) * CH)
                    # finish the FFN AllReduce of the previous layer for
                    # this chunk (it ran under cover of ~80us of compute)
                    if pending_ar[ch] is not None:
                        retrieve_ar(pending_ar[ch], ch)
                        pending_ar[ch] = None
                    rmsnorm_ch(hh, ch)

                    # q/k projections + rope for this chunk
                    for wt, dest in ((wq, qT), (wk, kT)):
                        for ot in range(NQT):
                            pool, tg = (psA, "A") if ot % 2 else (psC, "C")
                            ps = pool.tile([128, CH], f32, tag=tg, name="ps")
                            for dt in range(NDT):
                                nc.tensor.matmul(
                                    ps[:],
                                    wt[:, dt, ot * 128:(ot + 1) * 128],
                                    hh[:, dt, cs],
                                    start=(dt == 0), stop=(dt == NDT - 1),
                                )
                            if OPTS["rope_psum"]:
                                qsrc = ps
                            else:
                                qsrc = sp.tile([128, CH], bf, tag="sc1")
                                nc.scalar.activation(qsrc[:], ps[:], Act.Copy)
                            t_t = sp.tile([128, CH], bf, tag="sc2")
                            u_t = sp.tile([128, CH], bf, tag="sc3")
                            nc.vector.tensor_mul(t_t[:], qsrc[:], cosr[:, cs])
                            nc.vector.tensor_mul(u_t[:], qsrc[:], sinr[:, cs])
                            # rotate-half with sign via permutation matmul
                            us = psB.tile([128, CH], f32, tag="B")
                            nc.tensor.matmul(
                                us[:], rsg[:], u_t[:], start=True, stop=True
                            )
                            nc.vector.tensor_add(
                                dest[:, ot, cs], t_t[:], us[:]
                            )

                    # v projection for this chunk's lt tiles
                    for lt in range(ch * 4, ch * 4 + 4):
                        ls = slice(lt * 128, (lt + 1) * 128)
                        pool, tg = (psA, "A") if lt % 2 else (psC, "C")
                        ps = pool.tile([128, DQ], f32, tag=tg, name="psv")
                        for dt in range(NDT):
                            nc.tensor.matmul(
                                ps[:], hh[:, dt, ls], wv[:, dt, :],
                                start=(dt == 0), stop=(dt == NDT - 1),
                            )
                        nc.scalar.activation(
                            vS_r[:, lt, :, 0:64],
                            ps[:].rearrange("p (h e) -> p h e", e=64),
                            Act.Copy,
                        )

                    # attention for this chunk, all 8 local heads
                    nlts = 4 if ch == 0 else 8

                    def att_exp_mask_av(h8, pot, ss, lt, at_tag):
                        d0 = 128 * lt - CH * ch
                        v0 = max(0, d0)
                        at = sp.tile([128, CH], bf, tag=at_tag)
                        nc.scalar.activation(
                            at[:, v0:CH], ss[:, v0:CH], Act.Exp, scale=0.125
                        )
                        if 0 <= d0 < CH:
                            if OPTS["dmask_mult"]:
                                nc.vector.tensor_mul(
                                    at[:, d0:d0 + 128], at[:, d0:d0 + 128],
                                    dmask[:],
                                )
                            else:
                                nc.gpsimd.affine_select(
                                    at[:, d0:d0 + 128], at[:, d0:d0 + 128],
                                    pattern=[[1, 128]],
                                    compare_op=Alu.is_ge,
                                    fill=0.0, base=0,
                                    channel_multiplier=-1,
                                )
                        last = lt == nlts - 1
                        if last and v0 > 0:
                            nc.gpsimd.memset(at[:, 0:v0], 0.0)
                        o0 = 0 if last else v0
                        nc.tensor.matmul(
                            pot[:, o0:CH],
                            vS[:, lt, h8 * 65:(h8 + 1) * 65], at[:, o0:CH],
                            start=(lt == 0), stop=last,
                            skip_group_check=(o0 > 0),
                        )

                    def att_normalize(pot, po, pb):
                        # rc = 1/denominator, broadcast via PE, multiply
                        rc = rp.tile([1, CH], bf, tag="rbf")
                        if OPTS["lnexp_recip"]:
                            # 1/x = exp(-ln(x)); Ln and Exp share one
                            # activation table set, so no table reloads
                            rl = rp.tile([1, CH], f32, tag="rf32")
                            nc.scalar.activation(rl[:], pot[64:65, :], Act.Ln)
                            nc.scalar.activation(
                                rc[:], rl[:], Act.Exp, scale=-1.0
                            )
                        else:
                            rcf = rp.tile([1, CH], f32, tag="rf32")
                            nc.vector.reciprocal(rcf[:], pot[64:65, :])
                            nc.vector.tensor_copy(rc[:], rcf[:])
                        bps = psE.tile([128, CH], f32, tag="E")
                        nc.tensor.matmul(
                            bps[0:64, :], oner[:, 0:64], rc[:],
                            start=True, stop=True,
                        )
                        if OPTS["two_psum"]:
                            nc.vector.tensor_mul(
                                oT[po:po + 64, pb, cs], pot[0:64, :],
                                bps[0:64, :],
                            )
                        else:
                            bsb = sp.tile([64, CH], bf, tag="sc2")
                            nc.vector.tensor_copy(bsb[:], bps[0:64, :])
                            nc.vector.tensor_mul(
                                oT[po:po + 64, pb, cs], pot[0:64, :], bsb[:]
                            )

                    if OPTS["pack_scores"]:
                        # heads 2u/2u+1 issue adjacent 64-row score matmuls
                        # that run concurrently on disjoint PE row-tiles
                        for u in range(NH // 2):
                            potA = psC.tile([65, CH], f32, tag="C",
                                            name="potA")
                            potB = psC.tile([65, CH], f32, tag="C",
                                            name="potB")
                            for lt in range(nlts):
                                d0 = 128 * lt - CH * ch
                                v0 = max(0, d0)
                                ks = slice(lt * 128, (lt + 1) * 128)
                                qs_ = slice(ch * CH + v0, (ch + 1) * CH)
                                ssA = psA.tile([128, CH], f32, tag="A",
                                               name="ssA")
                                ssB = psB.tile([128, CH], f32, tag="B",
                                               name="ssB")
                                nc.tensor.matmul(
                                    ssA[:, v0:CH], kT[0:64, u, ks],
                                    qT[0:64, u, qs_], start=True, stop=True,
                                )
                                nc.tensor.matmul(
                                    ssB[:, v0:CH], kT[64:128, u, ks],
                                    qT[64:128, u, qs_], start=True, stop=True,
                                )
                                att_exp_mask_av(2 * u, potA, ssA, lt, "sc1")
                                att_exp_mask_av(2 * u + 1, potB, ssB, lt,
                                                "sc4")
                            att_normalize(potA, 0, u)
                            att_normalize(potB, 64, u)
                    else:
                        for h8 in range(NH):
                            pb = h8 // 2
                            po = (h8 % 2) * 64
                            pot = psC.tile([65, CH], f32, tag="C", name="pot")
                            for lt in range(nlts):
                                pool, tg = (psA, "A") if lt % 2 else (psB, "B")
                                d0 = 128 * lt - CH * ch
                                v0 = max(0, d0)
                                ss = pool.tile([128, CH], f32, tag=tg,
                                               name="ss")
                                nc.tensor.matmul(
                                    ss[:, v0:CH],
                                    kT[po:po + 64, pb,
                                       lt * 128:(lt + 1) * 128],
                                    qT[po:po + 64, pb,
                                       ch * CH + v0:(ch + 1) * CH],
                                    start=True, stop=True,
                                )
                                att_exp_mask_av(h8, pot, ss, lt, "sc1")
                            att_normalize(pot, po, pb)
                    # kick this chunk's AllGather; consumed at wo(ch) below
                    agi = dp.tile([128, NQT, CH], bf, tag=f"ag{ly}{ch}i",
                                  addr_space=adsp)
                    ago = dp.tile([2, 128, NQT, CH], bf, tag=f"ag{ly}{ch}o",
                                  addr_space=adsp)
                    nc.gpsimd.dma_start(agi[:], oT[:, :, cs])
                    nc.gpsimd.collective_compute(
                        "AllGather", Alu.bypass, replica_groups=groups,
                        ins=[agi.opt()], outs=[ago.opt()],
                    )
                    ag_bufs.append(ago)

                # ---- wo + FFN per chunk (AGs complete under attn/ffn) ----
                for ch in range(NCH):
                    cs = slice(ch * CH, (ch + 1) * CH)
                    ago = ag_bufs[ch]
                    pso = [
                        pools8[ot].tile(
                            [128, CH], f32, tag=tags8[ot], name=f"wops_{ot}"
                        )
                        for ot in range(NDT)
                    ]
                    for kt in range(NDT):
                        og = ws.tile([128, CH], bf, tag="og", bufs=2)
                        fetch_eng.dma_start(og[:], ago[kt // NQT, :, kt % NQT, :])
                        for ot in range(NDT):
                            nc.tensor.matmul(
                                pso[ot][:],
                                wo[:, kt, ot * 128:(ot + 1) * 128], og[:],
                                start=(kt == 0), stop=(kt == NDT - 1),
                            )
                    for ot in range(NDT):
                        nc.vector.tensor_add(
                            xT[:, ot, cs], xT[:, ot, cs], pso[ot][:]
                        )

                    # ---- FFN for this chunk ----
                    rmsnorm_ch(hh, ch)
                    # w1 -> silu -> g
                    for ftb in range(4):
                        pss = []
                        for j in range(4):
                            tagp = ("A", "A", "B", "B")[j]
                            pool = (psA, psA, psB, psB)[j]
                            pss.append(pool.tile(
                                [128, CH], f32, tag=tagp, name=f"ps1_{j}"
                            ))
                        for dh in range(2):
                            w1s = ws.tile([128, 4, 512], bf, tag="wms", bufs=2)
                            nc.sync.dma_start(
                                w1s[:],
                                w1_d[ly, :, 4 * dh:4 * dh + 4,
                                     ftb * 512:(ftb + 1) * 512],
                            )
                            for di in range(4):
                                dt = 4 * dh + di
                                for j in range(4):
                                    nc.tensor.matmul(
                                        pss[j][:],
                                        w1s[:, di, j * 128:(j + 1) * 128],
                                        hh[:, dt, cs],
                                        start=(dt == 0), stop=(dt == NDT - 1),
                                    )
                        for j in range(4):
                            if sim_mode:
                                sgt = sp.tile([128, CH], bf, tag="sc2",
                                              name="sgt")
                                nc.scalar.activation(
                                    sgt[:], pss[j][:], Act.Sigmoid
                                )
                                nc.vector.tensor_mul(
                                    g[:, ftb * 4 + j, cs], sgt[:], pss[j][:]
                                )
                            else:
                                nc.scalar.activation(
                                    g[:, ftb * 4 + j, cs], pss[j][:], Act.Silu
                                )
                    # w3 -> u = g * w3 (in place in g)
                    for ftb in range(4):
                        pss = []
                        for j in range(4):
                            tagp = ("A", "A", "B", "B")[j]
                            pool = (psA, psA, psB, psB)[j]
                            pss.append(pool.tile(
                                [128, CH], f32, tag=tagp, name=f"ps3_{j}"
                            ))
                        for dh in range(2):
                            w3s = ws.tile([128, 4, 512], bf, tag="wms", bufs=2)
                            nc.sync.dma_start(
                                w3s[:],
                                w3_d[ly, :, 4 * dh:4 * dh + 4,
                                     ftb * 512:(ftb + 1) * 512],
                            )
                            for di in range(4):
                                dt = 4 * dh + di
                                for j in range(4):
                                    nc.tensor.matmul(
                                        pss[j][:],
                                        w3s[:, di, j * 128:(j + 1) * 128],
                                        hh[:, dt, cs],
                                        start=(dt == 0), stop=(dt == NDT - 1),
                                    )
                        for j in range(4):
                            ft = ftb * 4 + j
                            nc.vector.tensor_mul(
                                g[:, ft, cs], g[:, ft, cs], pss[j][:]
                            )
                    # w2: 8 concurrent psums over all banks
                    pso = [
                        pools8[ot].tile(
                            [128, CH], f32, tag=tags8[ot], name=f"pso_{ot}"
                        )
                        for ot in range(NDT)
                    ]
                    for ftg in range(8):
                        w2s = ws.tile([128, 2, D], bf, tag="w2s", bufs=2)
                        nc.sync.dma_start(
                            w2s[:], w2_d[ly, :, 2 * ftg:2 * ftg + 2, :]
                        )
                        for fi in range(2):
                            ft = 2 * ftg + fi
                            for ot in range(NDT):
                                nc.tensor.matmul(
                                    pso[ot][:],
                                    w2s[:, fi, ot * 128:(ot + 1) * 128],
                                    g[:, ft, cs],
                                    start=(ft == 0), stop=(ft == NFT - 1),
                                )
                    # kick this chunk's AllReduce; retrieved next layer
                    inb = dp.tile([128, NDT, CH], ccdt, tag=f"ff{ly}{ch}i",
                                  addr_space=adsp)
                    outb = dp.tile([128, NDT, CH], ccdt, tag=f"ff{ly}{ch}o",
                                   addr_space=adsp)
                    for ot in range(NDT):
                        stg = st.tile([128, CH], ccdt, tag="stg")
                        if OPTS["dve_copies"]:
                            nc.vector.tensor_copy(stg[:], pso[ot][:])
                        else:
                            nc.scalar.activation(stg[:], pso[ot][:], Act.Copy)
                        nc.gpsimd.dma_start(inb[:, ot, :], stg[:])
                    nc.gpsimd.collective_compute(
                        "AllReduce", Alu.add, replica_groups=groups,
                        ins=[inb.opt()], outs=[outb.opt()],
                    )
                    pending_ar[ch] = outb

            # ---- final norm (* norm_w) and output, per chunk ----
            outsb = gp.tile([128, NDT, L], f32, tag="big")
            out_r = out_d[:].rearrange("(dt p) l -> p dt l", p=128)
            for ch in range(NCH):
                cs = slice(ch * CH, (ch + 1) * CH)
                retrieve_ar(pending_ar[ch], ch)
                pending_ar[ch] = None
                rmsnorm_ch(outsb, ch, with_nw=True)
                nc.sync.dma_start(out_r[:, :, cs], outsb[:, :, cs])

    nc.finalize()
    return nc


def _get_nc(pairs=4):
    if pairs not in _CACHE:
        _CACHE[pairs] = _build(pairs)
    return _CACHE[pairs]


def _rsign_const():
    # usigned = R2 @ u : usigned[0:32] = -u[32:64], usigned[32:64] = u[0:32]
    # per 64-block (two heads per 128-partition tile). Pass lhsT = R2.T.
    rh = np.zeros((64, 64), np.float32)
    rh[np.arange(32), np.arange(32) + 32] = -1.0
    rh[np.arange(32) + 32, np.arange(32)] = 1.0
    r2 = np.zeros((128, 128), np.float32)
    r2[0:64, 0:64] = rh
    r2[64:128, 64:128] = rh
    return np.ascontiguousarray(r2.T).astype(ml_dtypes.bfloat16)


def _dmask_const():
    # multiplicative causal mask for a diagonal 128x128 block of the
    # exp'd scores [key p, query j]: keep (1.0) where p <= j, else 0
    p = np.arange(128)
    m = (p[:, None] <= p[None, :]).astype(np.float32)
    return m.astype(ml_dtypes.bfloat16)


def _prep_core_inputs(inputs, b, t):
    """Host-side shard/layout prep for core (pair b, tp half t)."""
    f32 = np.float32
    bf = ml_dtypes.bfloat16
    x_processed = np.asarray(inputs["x_processed"], f32)
    boundaries = np.asarray(inputs["boundaries"], np.int32)
    x_residual = np.asarray(inputs["x_residual"], f32)
    cos = np.asarray(inputs["cos"], f32)
    sin = np.asarray(inputs["sin"], f32)
    wq = np.asarray(inputs["wq"], f32)
    wk = np.asarray(inputs["wk"], f32)
    wv = np.asarray(inputs["wv"], f32)
    wo = np.asarray(inputs["wo"], f32)
    w1 = np.asarray(inputs["w1"], f32)
    w2 = np.asarray(inputs["w2"], f32)
    w3 = np.asarray(inputs["w3"], f32)
    attn_norm_w = np.asarray(inputs["attn_norm_w"], f32)
    ffn_norm_w = np.asarray(inputs["ffn_norm_w"], f32)
    norm_w = np.asarray(inputs["norm_w"], f32)

    bnd = boundaries[b].astype(f32).copy()
    bnd[0] = 0.0  # match searchsorted-then-clip(>=0) semantics
    bnds = np.concatenate([bnd[1:], [np.float32(2 * L)]])

    def dtile(w, ncols):  # [D, ncols] -> [128, D//128, ncols]
        return np.ascontiguousarray(
            w.reshape(-1, 128, ncols).transpose(1, 0, 2)
        )

    qs = slice(t * DQ, (t + 1) * DQ)
    fs = slice(t * DF, (t + 1) * DF)
    wq_s = np.stack([
        dtile((attn_norm_w[l][:, None] * wq[l])[:, qs].astype(bf), DQ)
        for l in range(NL)
    ])
    wk_s = np.stack([
        dtile((attn_norm_w[l][:, None] * wk[l])[:, qs].astype(bf), DQ)
        for l in range(NL)
    ])
    wv_s = np.stack([
        dtile((attn_norm_w[l][:, None] * wv[l])[:, qs].astype(bf), DQ)
        for l in range(NL)
    ])
    wo_s = np.stack([dtile(wo[l].astype(bf), D) for l in range(NL)])
    w1_s = np.stack([
        dtile((ffn_norm_w[l][:, None] * w1[l])[:, fs].astype(bf), DF)
        for l in range(NL)
    ])
    w3_s = np.stack([
        dtile((ffn_norm_w[l][:, None] * w3[l])[:, fs].astype(bf), DF)
        for l in range(NL)
    ])
    w2_s = np.stack([dtile(w2[l][fs, :].astype(bf), D) for l in range(NL)])

    cosT = cos.T.astype(bf)  # [HD, L]
    sinT = sin.T.astype(bf)
    cos_rep = np.concatenate([cosT, cosT], axis=0)  # [128, L]
    sin_rep = np.concatenate([sinT, sinT], axis=0)

    xres_t = np.ascontiguousarray(
        x_residual[b].T.reshape(NDT, 128, L).transpose(1, 0, 2)
    )

    return {
        "xp": np.ascontiguousarray(x_processed[b]),
        "bnd": bnd[:, None],
        "bnds": bnds[:, None],
        "xres": xres_t,
        "cosr": np.ascontiguousarray(cos_rep),
        "sinr": np.ascontiguousarray(sin_rep),
        "wq": wq_s, "wk": wk_s, "wv": wv_s, "wo": wo_s,
        "w1": w1_s, "w3": w3_s, "w2": w2_s,
        "onec": np.ones((128, 1), bf),
        "rsg": _rsign_const(),
        "dmask": _dmask_const(),
        "epsc": np.full((1, 1), EPS, f32),
        "oner": np.ones((1, 128), bf),
        "nwr": norm_w[None, :].astype(bf),
    }


def kernel(**inputs) -> np.ndarray:
    from concourse.bass_utils import run_bass_kernel_spmd

    nc = _get_nc(4)
    in_maps = []
    for c in range(NCORES):
        in_maps.append(_prep_core_inputs(inputs, c // 2, c % 2))
    res = run_bass_kernel_spmd(nc, in_maps, list(range(NCORES)))
    out = np.empty((B, L, D), np.float32)
    for b in range(B):
        out[b] = res.results[2 * b]["out"].T
    return out
